# revision 19
# baseline (speedup 1.0000x reference)
"""Trainium2 Bass kernel for a 16-head linear-attention ("ALU") transformer block.

Reference computation (per row r of x, flattened over [B, N]):
    q  = x @ Wq.T                     # [R, 2048] -> 16 heads x 128
    g  = silu(x @ Wg.T)               # [R, 2048]
    e_h = silu(q_h @ (s*k_h).T)       # [R, 4096]   s = sqrt(128)
    o_h = e_h @ (s*v_h)               # [R, 128]
    out = (concat_h(o_h) * g) @ Wout.T

Strategy: pure data-parallel over the 4096 rows (512 rows/core, 8 cores, no
collectives). Weights are replicated; all operands are pre-transposed and cast
to bf16 on the host so every DMA is contiguous and every matmul contracts on
the partition axis. fp32 accumulation in PSUM throughout.
"""

import sys
import types

import numpy as np
import ml_dtypes

import concourse.bass as bass
import concourse.mybir as mybir
import concourse.tile as tile
from concourse import bacc
from concourse.bass_utils import run_bass_kernel_spmd

BF16 = mybir.dt.bfloat16
F32 = mybir.dt.float32

P = 128          # partitions / head dim
H = 16           # heads
CE = 16          # contraction chunks over E=2048
R = 512          # rows per core
MI = 32          # m-chunks per head (M=4096)
NCORES = 8
E = 2048
M = 4096
SCALE = float(np.float32(P) ** 0.5)

# Attention-phase energy groups: m-chunks per silu group (PSUM-bank limited:
# 2x3 banks energy double-buffer + 2x1 bank accumulators = 8 banks).
EGROUPS = [2] * 16
assert sum(EGROUPS) == MI


def install_ntff_hook():
    """Install the axon NTFF profiling hook that the stub `antenv` lacks."""
    import antenv

    if "antenv.axon_hooks" in sys.modules:
        return
    try:
        from trn_agent_boot.trn_boot import _ntff_profile_via_ctypes

        hook = _ntff_profile_via_ctypes("/opt/axon/libaxon_pjrt.so")
    except Exception:
        hook = None
    mod = types.ModuleType("antenv.axon_hooks")
    mod.get_axon_ntff_profile_hook = lambda: hook
    mod.set_axon_ntff_profile_hook = lambda h: None
    sys.modules["antenv.axon_hooks"] = mod
    antenv.axon_hooks = mod


def build_nc():
    nc = bacc.Bacc("TRN2", target_bir_lowering=False, debug=False, num_devices=NCORES)

    xt_d = nc.dram_tensor("xt", [P, CE, R], BF16, kind="ExternalInput").ap()
    wqt_d = nc.dram_tensor("wqt", [H, P, CE, P], BF16, kind="ExternalInput").ap()
    wgt_d = nc.dram_tensor("wgt", [H, P, CE, P], BF16, kind="ExternalInput").ap()
    kt_d = nc.dram_tensor("kt", [H, P, MI, P], BF16, kind="ExternalInput").ap()
    v_d = nc.dram_tensor("v", [H, P, MI, P], BF16, kind="ExternalInput").ap()
    wout_d = nc.dram_tensor("woutt", [4, P, CE, 512], BF16, kind="ExternalInput").ap()
    out_d = nc.dram_tensor("out", [R, E], F32, kind="ExternalOutput").ap()

    with tile.TileContext(nc) as tc:
        with (
            tc.tile_pool(name="const", bufs=1) as const,
            tc.tile_pool(name="wstream", bufs=2) as wstream,
            tc.tile_pool(name="kv", bufs=2) as kv,
            tc.tile_pool(name="esb", bufs=4) as esbp,
            tc.tile_pool(name="ysb", bufs=3) as ysbp,
            tc.tile_pool(name="wos", bufs=2) as wos,
            tc.tile_pool(name="acc", bufs=2, space="PSUM") as accp,
            tc.tile_pool(name="eps", bufs=3, space="PSUM") as epsp,
        ):
            xt = const.tile([P, CE, R], BF16, tag="xt")
            qt = const.tile([P, H, R], BF16, tag="qt")
            gt = const.tile([P, H, R], BF16, tag="gt")
            og = const.tile([P, H, R], BF16, tag="og")
            dz = const.tile([P, R], BF16, tag="dz")

            # PE warm-up: matmuls with no DMA dependency keep the PE busy
            # (and its HAM throughput ramp warm) until the first x bytes
            # land, so the real matmuls start at full rate.
            nc.gpsimd.memset(dz[:], 0.0)
            for _ in range(10):
                dps = epsp.tile([P, 2, R], F32, tag="eps", name="dps")
                nc.tensor.matmul(dps[:, 0, :], lhsT=dz[:, :P], rhs=dz[:],
                                 start=True, stop=True)

            # Prefetch head-0 Wq first, then x in 4 slices so the first Q
            # matmuls start as soon as the first slice lands.
            wq_first = wstream.tile([P, CE, P], BF16, tag="wq")
            nc.sync.dma_start(wq_first[:], wqt_d[0])
            for s in range(4):
                nc.sync.dma_start(xt[:, s * 4:(s + 1) * 4, :], xt_d[:, s * 4:(s + 1) * 4, :])

            def emit_q(h, wq_t=None):
                if wq_t is None:
                    wq_t = wstream.tile([P, CE, P], BF16, tag="wq")
                    nc.sync.dma_start(wq_t[:], wqt_d[h])
                ps = accp.tile([P, R], F32, tag="acc")
                for c in range(CE):
                    nc.tensor.matmul(
                        ps[:], lhsT=wq_t[:, c, :], rhs=xt[:, c, :],
                        start=(c == 0), stop=(c == CE - 1),
                    )
                nc.vector.tensor_copy(qt[:, h, :], ps[:])

            def emit_gate(j):
                wg_t = wstream.tile([P, CE, P], BF16, tag="wg")
                nc.sync.dma_start(wg_t[:], wgt_d[j])
                ps = accp.tile([P, R], F32, tag="acc")
                for c in range(CE):
                    nc.tensor.matmul(
                        ps[:], lhsT=wg_t[:, c, :], rhs=xt[:, c, :],
                        start=(c == 0), stop=(c == CE - 1),
                    )
                nc.scalar.activation(gt[:, j, :], ps[:], mybir.ActivationFunctionType.Silu)

            def emit_attn(h):
                kt_t = kv.tile([P, MI, P], BF16, tag="kt")
                nc.sync.dma_start(kt_t[:], kt_d[h])
                v_t = kv.tile([P, MI, P], BF16, tag="v")
                nc.sync.dma_start(v_t[:], v_d[h])

                ops = accp.tile([P, R], F32, tag="acc")
                i = 0
                for g in EGROUPS:
                    eps = epsp.tile([P, 2, R], F32, tag="eps")
                    for gi in range(g):
                        nc.tensor.matmul(
                            eps[:, gi, :], lhsT=kt_t[:, i + gi, :], rhs=qt[:, h, :],
                            start=True, stop=True,
                        )
                    esb = esbp.tile([P, 2, R], BF16, tag="esb")
                    nc.scalar.activation(
                        esb[:, :g, :], eps[:, :g, :], mybir.ActivationFunctionType.Silu
                    )
                    for gi in range(g):
                        nc.tensor.matmul(
                            ops[:], lhsT=v_t[:, i + gi, :], rhs=esb[:, gi, :],
                            start=(i + gi == 0), stop=(i + gi == MI - 1),
                        )
                    i += g
                nc.vector.tensor_mul(og[:, h, :], ops[:], gt[:, h, :])

            # Software pipeline: attention of head h is ACT(silu)-paced, so the
            # independent Q(h+2)/gate(h) matmul groups are emitted between heads
            # for the scheduler to fill TensorE gaps with.
            emit_q(0, wq_t=wq_first)
            emit_q(1)
            emit_gate(0)
            for h in range(H):
                emit_attn(h)
                if h + 2 < H:
                    emit_q(h + 2)
                if h + 1 < H:
                    emit_gate(h + 1)

            # ---- Output projection: out = (og)^T @ WoutT ----
            for n in range(4):
                wo_t = wos.tile([P, CE, 512], BF16, tag="wo")
                nc.sync.dma_start(wo_t[:], wout_d[n])
                for t in range(4):
                    last = (n == 3 and t == 3)
                    if not last:
                        if n == 0 and t == 0:
                            # First tile on an eps-pool psum: free of acc-ring
                            # coupling to og-mul(15), so its chunks hoist into
                            # head 15's slack and across the og[15] tail chain.
                            full = epsp.tile([P, 2, R], F32, tag="eps", name="oeps")
                            ps = full[:, 0, :]
                        else:
                            ps = accp.tile([P, R], F32, tag="acc")
                        for cv in range(CE):
                            nc.tensor.matmul(
                                ps[:], lhsT=og[:, cv, t * P:(t + 1) * P], rhs=wo_t[:, cv, :],
                                start=(cv == 0), stop=(cv == CE - 1),
                            )
                        ysb = ysbp.tile([P, 512], F32, tag="ysb")
                        nc.vector.tensor_copy(ysb[:], ps[:])
                        nc.sync.dma_start(
                            out_d[t * P:(t + 1) * P, n * 512:(n + 1) * 512], ysb[:]
                        )
                    else:
                        # Final tile: two column halves so the first half's
                        # copy+DMA overlap the second half's matmuls, keeping
                        # the post-last-matmul serial chain short.
                        for half in range(2):
                            ps = accp.tile([P, R // 2], F32, tag="acc")
                            for cv in range(CE):
                                nc.tensor.matmul(
                                    ps[:],
                                    lhsT=og[:, cv, t * P:(t + 1) * P],
                                    rhs=wo_t[:, cv, half * 256:(half + 1) * 256],
                                    start=(cv == 0), stop=(cv == CE - 1),
                                )
                            ysb = ysbp.tile([P, R // 2], F32, tag="ysb")
                            nc.vector.tensor_copy(ysb[:], ps[:])
                            nc.sync.dma_start(
                                out_d[t * P:(t + 1) * P,
                                      n * 512 + half * 256: n * 512 + (half + 1) * 256],
                                ysb[:],
                            )

    nc.compile()
    return nc


def prep_inputs(x, Wq, k_weight, v_weight, Wg, Wout):
    """Host-side: shard x, pre-transpose + bf16-cast all operands."""
    bf = ml_dtypes.bfloat16
    xf = np.ascontiguousarray(np.asarray(x, dtype=np.float32).reshape(NCORES * R, E))

    wqt = np.ascontiguousarray(
        np.asarray(Wq, np.float32).T.reshape(CE, P, H, P).transpose(2, 1, 0, 3)
    ).astype(bf)
    wgt = np.ascontiguousarray(
        np.asarray(Wg, np.float32).T.reshape(CE, P, H, P).transpose(2, 1, 0, 3)
    ).astype(bf)
    kt = np.ascontiguousarray(
        (np.asarray(k_weight, np.float32) * SCALE).T.reshape(H, P, MI, P)
    ).astype(bf)
    v = np.ascontiguousarray(
        (np.asarray(v_weight, np.float32) * SCALE).reshape(MI, P, H, P).transpose(2, 1, 0, 3)
    ).astype(bf)
    wout = np.ascontiguousarray(
        np.asarray(Wout, np.float32).T.reshape(CE, P, 4, 512).transpose(2, 1, 0, 3)
    ).astype(bf)

    in_maps = []
    for c in range(NCORES):
        shard = xf[c * R:(c + 1) * R]  # [512, 2048]
        xt = np.ascontiguousarray(shard.T.reshape(CE, P, R).transpose(1, 0, 2)).astype(bf)
        in_maps.append(
            {"xt": xt, "wqt": wqt, "wgt": wgt, "kt": kt, "v": v, "woutt": wout}
        )
    return in_maps


_NC_CACHE = None


def get_nc():
    global _NC_CACHE
    if _NC_CACHE is None:
        _NC_CACHE = build_nc()
    return _NC_CACHE


def run(in_maps, trace=False):
    if trace:
        install_ntff_hook()
    return run_bass_kernel_spmd(
        get_nc(), in_maps, core_ids=list(range(NCORES)), trace=trace
    )


def kernel(x, Wq, k_weight, v_weight, Wg, Wout):
    B, N, Ein = x.shape
    in_maps = prep_inputs(x, Wq, k_weight, v_weight, Wg, Wout)
    res = run(in_maps, trace=False)
    out = np.concatenate([res.results[i]["out"] for i in range(NCORES)], axis=0)
    return out.reshape(B, N, Ein).astype(np.float32)

